# revision 12
# baseline (speedup 1.0000x reference)
"""Trainium2 Bass kernel for the AttentionBlock problem.

Fixed problem shape: x [4, 64, 64, 64] fp32, GroupNorm(32 groups) ->
1x1 conv Q/K/V -> softmax(Q^T K / 8) -> V @ attn^T -> 1x1 conv + residual.

Sharding: 8 cores, core = 2*batch + query_half. Each core holds its batch's
full x (for K/V) and computes outputs for its 2048-query half.

Layout strategy (per core):
  - x, xn, K, Q, V live as [c=64 partitions, n free].
  - Scores are computed TRANSPOSED: S_T[k,q] = K_blk^T Q (contract c on
    partitions); the softmax denominator comes free from a ones-row appended
    to V^T during the PV matmul (no cross-partition reductions needed).
  - exp() runs on ScalarE directly out of PSUM in 1024-wide ops, no max
    subtraction (scores are O(+-10) here; exp stays well inside fp32 range).
  - PE stationary operands are switched only twice per key block (K_blk for
    4 score matmuls, V^T_blk for 4 PV matmuls) — a stationary switch costs
    ~290 ns of drain+reload on this hardware.
  - Q and K are produced by ONE matmul pass with a stacked [Wq^T | Wk^T]
    stationary; V^T blocks and the output-projection transposes go through
    the DMA xbar transpose (bf16), keeping them off the PE.
"""

import numpy as np
import ml_dtypes

import concourse.bass as bass
import concourse.mybir as mybir
import concourse.tile as tile
from concourse.tile_rust import add_dep_helper
from concourse.vector_clock import ScopedClock

B, C, H, W = 4, 64, 64, 64
N = H * W            # 4096
NQ = N // 2          # queries per core
EPS = 1e-5
KB = 32              # key blocks of 128
WARMUP_REPS = 112    # ~48us of PE warmup at the cold 1.2 GHz clock
F32 = mybir.dt.float32
BF16 = mybir.dt.bfloat16
AF = mybir.ActivationFunctionType
ALU = mybir.AluOpType


# ---------------------------------------------------------------------------
# This container's walrus codegen rejects >1 sync wait on one instruction
# ("Too many sync wait commands") — split extra waits onto preceding same-
# engine NOPs (engines execute in order, so semantics are preserved), and do
# the same for the TileContext tail drain.
def _install_drain_patch():
    if getattr(tile.TileContext, "_drain_patch_installed", False):
        return

    orig_commit = tile.TileContext._commit_instruction

    def _split_commit(self, inst, lazy_reg_writes=True):
        si = getattr(inst, "sync_info", None)
        if (
            si is not None
            and len(si.on_wait) > 1
            and inst.engine != mybir.EngineType.Unassigned
        ):
            waits = list(si.on_wait)
            inst.sync_info = mybir.SyncInfo(
                on_wait=waits[-1:], on_update=list(si.on_update)
            )
            for w in waits[:-1]:
                nop = mybir.InstNoOp(
                    name=self.nc.get_next_instruction_name(),
                    sync_info=mybir.SyncInfo(on_wait=[w], on_update=[]),
                    bass_nofuse=True,
                    engine=inst.engine,
                )
                orig_commit(self, nop, lazy_reg_writes=False)
        orig_commit(self, inst, lazy_reg_writes)

    def _patched(self, tick_clock, wait_clock):
        nc = self.nc
        drain_inst = nc.sync.drain()
        wait_clock.add_sem_waits(
            drain_inst.ins, ScopedClock({None: tick_clock.global_clock})
        )
        si = drain_inst.ins.sync_info
        if si is not None and len(si.on_wait) > 1:
            waits = list(si.on_wait)
            drain_inst.ins.sync_info = mybir.SyncInfo(
                on_wait=waits[:1], on_update=list(si.on_update)
            )
            for i in range(1, len(waits)):
                extra = nc.sync.drain()
                extra.ins.sync_info = mybir.SyncInfo(
                    on_wait=waits[i : i + 1], on_update=[]
                )
        nc.all_engine_barrier()
        assert self.sems is not None
        popped = nc._tile_sem_poison_stack.pop()
        assert popped is self._sem_poison
        nc.clear_and_free_semaphores(list(self.sems.allocated().values()))
        nc.all_engine_barrier()

    tile.TileContext._commit_instruction = _split_commit
    tile.TileContext._drain_and_barrier = _patched
    tile.TileContext._drain_patch_installed = True


def build_nc():
    _install_drain_patch()
    nc = bass.Bass()

    # per-core data
    x_d = nc.dram_tensor("x", [C, N], F32, kind="ExternalInput")
    xq_d = nc.dram_tensor("xq", [C, NQ], F32, kind="ExternalInput")
    xt_d = nc.dram_tensor("xt", [NQ, C], F32, kind="ExternalInput")
    # replicated weights / constants
    wqk_d = nc.dram_tensor("w_qk", [C, 2 * C], BF16, kind="ExternalInput")
    wv_d = nc.dram_tensor("wv_t", [C, C], BF16, kind="ExternalInput")
    waug_d = nc.dram_tensor("w_aug", [C + 1, C + 1], BF16, kind="ExternalInput")
    pair_d = nc.dram_tensor("pairmat", [C, C], F32, kind="ExternalInput")
    # gbias columns: 0 gamma, 1 beta, 2 [bq;bk] stacked, 3 bv
    gb_d = nc.dram_tensor("gbias", [2 * C, 4], F32, kind="ExternalInput")
    bo_d = nc.dram_tensor("bo_bc", [128, C], F32, kind="ExternalInput")
    y_d = nc.dram_tensor("y", [NQ, C], F32, kind="ExternalOutput")

    with tile.TileContext(nc) as tc:
        with (
            tc.tile_pool(name="const", bufs=1) as const,
            tc.tile_pool(name="big", bufs=1) as big,
            tc.tile_pool(name="stats", bufs=2) as stats,
            tc.tile_pool(name="pt", bufs=4) as ptp,
            tc.tile_pool(name="tail", bufs=2) as tailp,
            tc.tile_pool(name="yp", bufs=3) as yp,
            tc.tile_pool(name="xtp", bufs=3) as xtp,
            tc.tile_pool(name="sps", bufs=2, space="PSUM") as sps,
            tc.tile_pool(name="ops", bufs=4, space="PSUM") as ops,
        ):
            # ---- load constants
            wqk = const.tile([C, 2 * C], BF16, tag="wqk")
            wv = const.tile([C, C], BF16, tag="wv")
            waug = const.tile([C + 1, C + 1], BF16, tag="waug")
            pair = const.tile([C, C], F32, tag="pair")
            gb = const.tile([2 * C, 4], F32, tag="gb")
            bo_bc = const.tile([128, C], F32, tag="bo")
            nc.sync.dma_start(out=wqk, in_=wqk_d[:, :])
            nc.sync.dma_start(out=wv, in_=wv_d[:, :])
            nc.sync.dma_start(out=waug, in_=waug_d[:, :])
            nc.sync.dma_start(out=pair, in_=pair_d[:, :])
            nc.sync.dma_start(out=gb, in_=gb_d[:, :])
            nc.sync.dma_start(out=bo_bc, in_=bo_d[:, :])
            gamma = gb[:C, 0:1]
            beta = gb[:C, 1:2]
            bqk_col = gb[:, 2:3]
            bv_col = gb[:C, 3:4]

            # ---- PE clock warmup: ~48us of back-to-back same-weight
            # matmuls. The PE clock ramps 1.2->2.4 GHz only after ~46us of
            # sustained activity and then stays warm; this burst overlaps the
            # DMA/GroupNorm preamble so the attention loop runs warm.
            warm_sb = const.tile([128, 512], BF16, tag="warm")
            nc.vector.memset(warm_sb, 0.0)
            for i in range(WARMUP_REPS):
                wp = sps.tile([128, 512], F32, tag="sps", name=f"warm{i}")
                nc.tensor.matmul(
                    out=wp, lhsT=warm_sb[:, 0:128], rhs=warm_sb,
                    start=True, stop=True,
                )

            # ---- load x; bn_stats per 512-chunk as chunks arrive
            x_sb = big.tile([C, N], F32, tag="x")
            xq_sb = big.tile([C, NQ], F32, tag="xq")
            st = stats.tile([C, 8, 6], F32, tag="bnst")
            for j in range(8):
                sl = bass.ts(j, 512)
                nc.sync.dma_start(out=x_sb[:, sl], in_=x_d[:, sl])
                nc.vector.bn_stats(out=st[:, j, :], in_=x_sb[:, sl])
            for j in range(4):
                sl = bass.ts(j, 512)
                nc.sync.dma_start(out=xq_sb[:, sl], in_=xq_d[:, sl])
            mv = stats.tile([C, 2], F32, tag="mv")
            nc.vector.bn_aggr(out=mv, in_=st)
            # me2 = [mean, var + mean^2] per channel
            me2 = stats.tile([C, 2], F32, tag="me2")
            nc.vector.tensor_copy(out=me2[:, 0:1], in_=mv[:, 0:1])
            m2 = stats.tile([C, 1], F32, tag="m2")
            nc.vector.tensor_mul(out=m2, in0=mv[:, 0:1], in1=mv[:, 0:1])
            nc.vector.tensor_add(out=me2[:, 1:2], in0=mv[:, 1:2], in1=m2)
            # group (channel-pair) means of [mean, E[x^2]] via tiny matmul
            gps = ops.tile([C, 2], F32, tag="o")
            nc.tensor.matmul(out=gps, lhsT=pair, rhs=me2, start=True, stop=True)
            mean_g = stats.tile([C, 1], F32, tag="meang")
            nc.vector.tensor_copy(out=mean_g, in_=gps[:, 0:1])
            varg = stats.tile([C, 1], F32, tag="varg")
            nc.vector.tensor_mul(out=varg, in0=mean_g, in1=mean_g)
            nc.vector.tensor_tensor(
                out=varg, in0=gps[:, 1:2], in1=varg, op=ALU.subtract
            )
            # rstd = 1/sqrt(var+eps);  s = rstd*gamma;  t = beta - mean*s
            eps_t = stats.tile([C, 1], F32, tag="eps")
            nc.vector.memset(eps_t, EPS)
            nc.scalar.activation(out=varg, in_=varg, func=AF.Sqrt, bias=eps_t)
            rstd = stats.tile([C, 1], F32, tag="rstd")
            nc.vector.reciprocal(out=rstd, in_=varg)
            s_col = stats.tile([C, 1], F32, tag="scol")
            nc.vector.tensor_mul(out=s_col, in0=rstd, in1=gamma)
            t_col = stats.tile([C, 1], F32, tag="tcol")
            nc.vector.tensor_mul(out=t_col, in0=mean_g, in1=s_col)
            nc.vector.tensor_tensor(out=t_col, in0=beta, in1=t_col, op=ALU.subtract)

            # ---- normalized activations (bf16)
            xn = big.tile([C, N], BF16, tag="xn")
            xnq = big.tile([C, NQ], BF16, tag="xnq")
            for j in range(8):
                sl = bass.ts(j, 512)
                nc.vector.tensor_scalar(
                    out=xn[:, sl], in0=x_sb[:, sl], scalar1=s_col, scalar2=t_col,
                    op0=ALU.mult, op1=ALU.add,
                )
            for j in range(4):
                sl = bass.ts(j, 512)
                nc.vector.tensor_scalar(
                    out=xnq[:, sl], in0=xq_sb[:, sl], scalar1=s_col, scalar2=t_col,
                    op0=ALU.mult, op1=ALU.add,
                )

            # ---- QK fused pass over xn (rows 0:64 = Q+bq, 64:128 = K+bk),
            #      V pass, all with one stationary each
            k_sb = big.tile([C, N], BF16, tag="k")
            q_sb = big.tile([C, NQ], BF16, tag="q")
            v_sb = big.tile([C, N], BF16, tag="v")
            # one wqk-stationary run: 8 chunks of xn (K rows) + 4 of xnq (Q rows)
            for j in range(8):
                sl = bass.ts(j, 512)
                ps = sps.tile([128, 512], F32, tag="sps", name=f"qk{j}")
                nc.tensor.matmul(out=ps, lhsT=wqk, rhs=xn[:, sl], start=True, stop=True)
                nc.vector.tensor_scalar(
                    out=k_sb[:, sl], in0=ps[64:128, :],
                    scalar1=bqk_col[64:128], scalar2=None, op0=ALU.add,
                )
            for j in range(4):
                sl = bass.ts(j, 512)
                ps = sps.tile([128, 512], F32, tag="sps", name=f"qq{j}")
                nc.tensor.matmul(out=ps, lhsT=wqk, rhs=xnq[:, sl], start=True, stop=True)
                nc.vector.tensor_scalar(
                    out=q_sb[:, sl], in0=ps[0:64, :],
                    scalar1=bqk_col[0:64], scalar2=None, op0=ALU.add,
                )
            # then one wv-stationary run
            for j in range(8):
                sl = bass.ts(j, 512)
                ps = sps.tile([64, 512], F32, tag="sps", name=f"v{j}")
                nc.tensor.matmul(out=ps, lhsT=wv, rhs=xn[:, sl], start=True, stop=True)
                nc.vector.tensor_scalar(
                    out=v_sb[:, sl], in0=ps,
                    scalar1=bv_col, scalar2=None, op0=ALU.add,
                )

            # ---- V^T blocks [128, 65] with ones column, via DMA xbar
            # per-block stride padded to 128 elements: the xbar transpose
            # needs 128B-aligned destination offsets. One call transposes all
            # of V: out[p, kb, c] = V^T[kb*128 + p, c].
            vt = big.tile([128, KB, 128], BF16, tag="vt")
            nc.sync.dma_start_transpose(out=vt[:, :, 0:C], in_=v_sb)
            nc.vector.memset(vt[:, :, C : C + 1], 1.0)

            # ---- main attention loop
            o_tiles = [
                ops.tile([C + 1, 512], F32, tag="o", name=f"o{qc}")
                for qc in range(4)
            ]
            # software-pipelined: PV for block kb-1 is emitted after the score
            # matmuls for block kb, so the PE does 4 same-stationary matmuls
            # per weight switch and exp(kb-1) has a full block to finish.
            def emit_pv(kb, p2, after):
                for qc in range(4):
                    mm = nc.tensor.matmul(
                        out=o_tiles[qc], lhsT=vt[:, kb, 0 : C + 1],
                        rhs=p2[qc // 2][:, (qc % 2) * 512 : (qc % 2 + 1) * 512],
                        start=(kb == 0), stop=(kb == KB - 1),
                        skip_group_check=True,
                    )
                    if qc == 0 and after is not None:
                        # keep the PE stream in same-stationary runs of 4:
                        # PV(kb-1) only after the last score matmul of kb
                        add_dep_helper(
                            mm.ins, after.ins, sync=False,
                            reason="group PE same-stationary runs",
                        )

            prev = None
            for kb in range(KB):
                kblk = k_sb[:, bass.ts(kb, 128)]
                s2 = []
                last_s = None
                for h in range(2):
                    sp = sps.tile([128, 1024], F32, tag="sps", name=f"s{kb}_{h}")
                    nc.tensor.matmul(
                        out=sp[:, 0:512], lhsT=kblk,
                        rhs=q_sb[:, bass.ds(h * 1024, 512)],
                        start=True, stop=True,
                    )
                    last_s = nc.tensor.matmul(
                        out=sp[:, 512:1024], lhsT=kblk,
                        rhs=q_sb[:, bass.ds(h * 1024 + 512, 512)],
                        start=True, stop=True,
                    )
                    s2.append(sp)
                p2 = []
                for h in range(2):
                    p = ptp.tile([128, 1024], BF16, tag="p", name=f"p{kb}_{h}")
                    nc.scalar.activation(out=p, in_=s2[h], func=AF.Exp, scale=0.125)
                    p2.append(p)
                if prev is not None:
                    emit_pv(kb - 1, prev, last_s)
                prev = p2
            emit_pv(KB - 1, prev, None)

            # ---- tail: project through augmented Wo, DMA-transpose,
            #      normalize by denominator, add residual + bo, store
            z_all = tailp.tile([80, 2048], BF16, tag="z", bufs=1)
            for qc in range(4):
                ou = tailp.tile([C + 1, 512], BF16, tag="ou")
                nc.vector.tensor_copy(out=ou, in_=o_tiles[qc])
                z_ps = sps.tile([C + 1, 512], F32, tag="sps", name=f"z{qc}")
                nc.tensor.matmul(out=z_ps, lhsT=waug, rhs=ou, start=True, stop=True)
                nc.vector.tensor_copy(
                    out=z_all[0 : C + 1, bass.ts(qc, 512)], in_=z_ps
                )
            zt_all = tailp.tile([128, 16, 128], BF16, tag="zt", bufs=1)
            nc.sync.dma_start_transpose(out=zt_all[:, :, 0:80], in_=z_all)
            for j in range(16):
                qrow = j * 128
                xt_t = xtp.tile([128, C], F32, tag="xt")
                nc.sync.dma_start(out=xt_t, in_=xt_d[qrow : qrow + 128, :])
                r = yp.tile([128, 1], F32, tag="r")
                nc.vector.reciprocal(out=r, in_=zt_all[:, j, C : C + 1])
                y1 = yp.tile([128, C], F32, tag="y1")
                nc.vector.scalar_tensor_tensor(
                    out=y1, in0=zt_all[:, j, 0:C], scalar=r, in1=xt_t,
                    op0=ALU.mult, op1=ALU.add,
                )
                y2 = yp.tile([128, C], F32, tag="y2")
                nc.vector.tensor_add(out=y2, in0=y1, in1=bo_bc)
                nc.sync.dma_start(out=y_d[qrow : qrow + 128, :], in_=y2)
    return nc


_NC = None


def _get_nc():
    global _NC
    if _NC is None:
        _NC = build_nc()
    return _NC


def _prep_maps(x, Wq, bq, Wk, bk, Wv, bv, Wo, bo, gamma, beta):
    bf = ml_dtypes.bfloat16
    w_qk = np.concatenate([Wq.T, Wk.T], axis=1).astype(bf)  # [cin, 2c]
    wv_t = np.ascontiguousarray(Wv.T).astype(bf)
    w_aug = np.zeros((C + 1, C + 1), np.float32)
    w_aug[:C, :C] = Wo.T
    w_aug[C, C] = 1.0
    w_aug = w_aug.astype(bf)
    pairmat = np.zeros((C, C), np.float32)
    for k in range(C):
        for m in range(C):
            if k // 2 == m // 2:
                pairmat[k, m] = 0.5
    gbias = np.zeros((2 * C, 4), np.float32)
    gbias[:C, 0] = gamma
    gbias[:C, 1] = beta
    gbias[:, 2] = np.concatenate([bq, bk])
    gbias[:C, 3] = bv
    bo_bc = np.tile(bo[None, :], (128, 1)).astype(np.float32)

    shared = dict(
        w_qk=w_qk, wv_t=wv_t, w_aug=w_aug, pairmat=pairmat, gbias=gbias,
        bo_bc=bo_bc,
    )
    in_maps = []
    for core in range(8):
        b, half = core // 2, core % 2
        xm = np.ascontiguousarray(x[b].reshape(C, N)).astype(np.float32)
        xqm = np.ascontiguousarray(xm[:, half * NQ : (half + 1) * NQ])
        xtm = np.ascontiguousarray(xm.T[half * NQ : (half + 1) * NQ, :])
        in_maps.append(dict(shared, x=xm, xq=xqm, xt=xtm))
    return in_maps


def run(inputs, trace=False):
    from concourse.bass_utils import run_bass_kernel_spmd

    inputs = {k: np.asarray(v) for k, v in inputs.items()}
    nc = _get_nc()
    in_maps = _prep_maps(**inputs)
    res = run_bass_kernel_spmd(
        nc, in_maps, core_ids=list(range(8)), trace=trace
    )
    out = np.empty((B, C, N), np.float32)
    for core in range(8):
        b, half = core // 2, core % 2
        out[b][:, half * NQ : (half + 1) * NQ] = res.results[core]["y"].T
    return out.reshape(B, C, H, W), res


def kernel(**inputs):
    out, _ = run(inputs, trace=False)
    return out
